# revision 3
# baseline (speedup 1.0000x reference)
"""Trainium2 Bass kernel for the LSTM-style encoder (B=256,T=256,E=512,Z=256,H=1024).

Strategy:
  - Data-parallel over batch: B=256 -> 32 rows per each of 8 cores; weights
    replicated. The sequential time loop stays local per shard.
  - Fully transposed on-chip layout: hidden/gate dim on SBUF partitions,
    batch on the free dim. Recurrent GEMM computed as gates.T = Wh @ h.T
    with Wh.T tiles (bf16) stationary, h.T (bf16) as the moving operand.
  - Input projections (Wi/Wz GEMMs) fused per time-chunk into SBUF; the
    z-path is pre-activated into u_z = sigmoid(l)*tanh(zhat) once per chunk.
  - Biases (bi+bh) folded into the gd eviction via tensor_scalar_add; bz
    folded into the z activations. Activations/elementwise in fp32.
"""

import os
import numpy as np
import ml_dtypes
from contextlib import ExitStack

import concourse.bacc as bacc
import concourse.bass as bass
import concourse.mybir as mybir
from concourse import tile
from concourse import bass_utils

BF16 = mybir.dt.bfloat16
F32 = mybir.dt.float32
AF = mybir.ActivationFunctionType

B, T, E, Z, H = 256, 256, 512, 256, 1024
NCORES = 8
BL = B // NCORES          # 32 batch rows per core
TC = 4                    # timesteps per chunk
KE, KZ, KH = E // 128, Z // 128, H // 128   # 4, 2, 8 contraction tiles
M4 = 4 * H // 128         # 32 output tiles of the 4H gate dim
M2 = 2 * H // 128         # 16 output tiles of the 2H z-gate dim

LAST_EXEC_NS = None
_CACHE = {}


def _build(t_steps=T):
    nch = t_steps // TC
    nc = bacc.Bacc("TRN2", target_bir_lowering=False, debug=False)

    ind = nc.dram_tensor("ind", [KE, 128, t_steps, BL], BF16, kind="ExternalInput").ap()
    inz = nc.dram_tensor("inz", [KZ, 128, t_steps, BL], BF16, kind="ExternalInput").ap()
    wit = nc.dram_tensor("wit", [KE, 128, 4 * H], BF16, kind="ExternalInput").ap()
    wht = nc.dram_tensor("wht", [KH, 128, 4 * H], BF16, kind="ExternalInput").ap()
    wzt = nc.dram_tensor("wzt", [KZ, 128, 2 * H], BF16, kind="ExternalInput").ap()
    bhi = nc.dram_tensor("bhi", [128, M4], F32, kind="ExternalInput").ap()
    bzt = nc.dram_tensor("bzt", [128, M2], F32, kind="ExternalInput").ap()
    h0 = nc.dram_tensor("h0", [128, KH, BL], F32, kind="ExternalInput").ap()
    c0 = nc.dram_tensor("c0", [128, KH, BL], F32, kind="ExternalInput").ap()
    hs = nc.dram_tensor("hs", [128, KH, t_steps, BL], F32, kind="ExternalOutput").ap()
    cs = nc.dram_tensor("cs", [128, KH, t_steps, BL], F32, kind="ExternalOutput").ap()

    with tile.TileContext(nc) as tc:
        with ExitStack() as ctx:
            wp = ctx.enter_context(tc.tile_pool(name="w", bufs=1))
            psum = ctx.enter_context(tc.tile_pool(name="psum", bufs=4, space="PSUM"))
            psumr = ctx.enter_context(tc.tile_pool(name="psumr", bufs=4, space="PSUM"))
            gdp = ctx.enter_context(tc.tile_pool(name="gd", bufs=2))
            uzp = ctx.enter_context(tc.tile_pool(name="uz", bufs=2))
            inp = ctx.enter_context(tc.tile_pool(name="inp", bufs=2))
            actp = ctx.enter_context(tc.tile_pool(name="act", bufs=6))
            hcp = ctx.enter_context(tc.tile_pool(name="hc", bufs=3))
            stg = ctx.enter_context(tc.tile_pool(name="stg", bufs=2))

            # ---- resident weights
            wht_sb = wp.tile([128, KH, 4 * H], BF16, tag="wht")
            for k in range(KH):
                nc.sync.dma_start(wht_sb[:, k, :], wht[k])
            wit_sb = wp.tile([128, KE, 4 * H], BF16, tag="wit")
            for k in range(KE):
                nc.sync.dma_start(wit_sb[:, k, :], wit[k])
            wzt_sb = wp.tile([128, KZ, 2 * H], BF16, tag="wzt")
            for k in range(KZ):
                nc.sync.dma_start(wzt_sb[:, k, :], wzt[k])
            bhi_sb = wp.tile([128, M4], F32, tag="bhi")
            nc.sync.dma_start(bhi_sb[:], bhi)
            bzt_sb = wp.tile([128, M2], F32, tag="bzt")
            nc.sync.dma_start(bzt_sb[:], bzt)

            # ---- carry state (h.T, c.T) in [128, KH, BL] tiling
            hT = hcp.tile([128, KH, BL], F32, tag="hT")
            nc.sync.dma_start(hT[:], h0)
            cT = hcp.tile([128, KH, BL], F32, tag="cT")
            nc.sync.dma_start(cT[:], c0)

            NF = TC * BL  # free size of one chunk (t-major, b-minor)

            for ch in range(nch):
                t0 = ch * TC
                # ---- chunk input DMA
                ind_sb = inp.tile([128, KE, TC, BL], BF16, tag="ind")
                for k in range(KE):
                    nc.sync.dma_start(ind_sb[:, k, :, :], ind[k, :, t0:t0 + TC, :])
                inz_sb = inp.tile([128, KZ, TC, BL], BF16, tag="inz")
                for k in range(KZ):
                    nc.sync.dma_start(inz_sb[:, k, :, :], inz[k, :, t0:t0 + TC, :])

                # ---- phase 1a: gd.T = Wi @ x.T + (bi+bh), per 128-row gate tile
                gd_sb = gdp.tile([128, M4, TC, BL], F32, tag="gd")
                for m in range(M4):
                    ps = psum.tile([128, TC, BL], F32, tag="ps")
                    for k in range(KE):
                        nc.tensor.matmul(
                            ps[:],
                            wit_sb[:, k, 128 * m:128 * (m + 1)],
                            ind_sb[:, k, :, :],
                            start=(k == 0), stop=(k == KE - 1),
                        )
                    nc.vector.tensor_scalar_add(gd_sb[:, m, :, :], ps[:], bhi_sb[:, m:m + 1])

                # ---- phase 1b: u_z.T = sigmoid(l.T) * tanh(zhat.T)
                uz_sb = uzp.tile([128, KH, TC, BL], F32, tag="uz")
                for j in range(KH):
                    psl = psum.tile([128, TC, BL], F32, tag="ps")
                    for k in range(KZ):
                        nc.tensor.matmul(
                            psl[:],
                            wzt_sb[:, k, 128 * j:128 * (j + 1)],
                            inz_sb[:, k, :, :],
                            start=(k == 0), stop=(k == KZ - 1),
                        )
                    sl = actp.tile([128, TC, BL], F32, tag="zact")
                    nc.scalar.activation(sl[:], psl[:], AF.Sigmoid, bias=bzt_sb[:, j:j + 1])

                    psz = psum.tile([128, TC, BL], F32, tag="ps")
                    jm = KH + j
                    for k in range(KZ):
                        nc.tensor.matmul(
                            psz[:],
                            wzt_sb[:, k, 128 * jm:128 * (jm + 1)],
                            inz_sb[:, k, :, :],
                            start=(k == 0), stop=(k == KZ - 1),
                        )
                    tz = actp.tile([128, TC, BL], F32, tag="zact")
                    nc.scalar.activation(tz[:], psz[:], AF.Tanh, bias=bzt_sb[:, jm:jm + 1])
                    nc.vector.tensor_mul(uz_sb[:, j, :, :], sl[:], tz[:])

                # ---- recurrence over the chunk
                hs_st = stg.tile([128, KH, TC, BL], F32, tag="hs_st")
                cs_st = stg.tile([128, KH, TC, BL], F32, tag="cs_st")
                for tt in range(TC):
                    hbf = hcp.tile([128, KH, BL], BF16, tag="hbf")
                    nc.vector.tensor_copy(hbf[:], hT[:])

                    gacts = []
                    for g, fn in enumerate((AF.Sigmoid, AF.Sigmoid, AF.Tanh, AF.Sigmoid)):
                        ps = psumr.tile([128, KH, BL], F32, tag="psr")
                        for mm in range(KH):
                            m = KH * g + mm
                            for k in range(KH):
                                nc.tensor.matmul(
                                    ps[:, mm, :],
                                    wht_sb[:, k, 128 * m:128 * (m + 1)],
                                    hbf[:, k, :],
                                    start=(k == 0), stop=(k == KH - 1),
                                )
                        nc.vector.tensor_add(ps[:], ps[:], gd_sb[:, KH * g:KH * (g + 1), tt, :])
                        a = actp.tile([128, KH, BL], F32, tag="gact")
                        nc.scalar.activation(a[:], ps[:], fn)
                        gacts.append(a)

                    si, sf, tg, so = gacts
                    t1 = actp.tile([128, KH, BL], F32, tag="tmp")
                    nc.vector.tensor_mul(t1[:], sf[:], cT[:])
                    t2 = actp.tile([128, KH, BL], F32, tag="tmp")
                    nc.vector.tensor_mul(t2[:], si[:], tg[:])
                    t3 = actp.tile([128, KH, BL], F32, tag="tmp")
                    nc.vector.tensor_add(t3[:], t1[:], t2[:])
                    cT = hcp.tile([128, KH, BL], F32, tag="cT")
                    nc.vector.tensor_add(cT[:], t3[:], uz_sb[:, :, tt, :])

                    tch = actp.tile([128, KH, BL], F32, tag="tmp")
                    nc.scalar.activation(tch[:], cT[:], AF.Tanh)
                    hT = hcp.tile([128, KH, BL], F32, tag="hT")
                    nc.vector.tensor_mul(hT[:], so[:], tch[:])

                    nc.vector.tensor_copy(hs_st[:, :, tt, :], hT[:])
                    nc.vector.tensor_copy(cs_st[:, :, tt, :], cT[:])

                nc.sync.dma_start(hs[:, :, t0:t0 + TC, :], hs_st[:])
                nc.sync.dma_start(cs[:, :, t0:t0 + TC, :], cs_st[:])

    nc.compile()
    return nc


def _get_nc(t_steps=T):
    if t_steps not in _CACHE:
        _CACHE[t_steps] = _build(t_steps)
    return _CACHE[t_steps]


def _prep_shared(Wi, bi, Wh, bh, Wz, bz):
    bf = ml_dtypes.bfloat16
    wit = np.ascontiguousarray(Wi.T.reshape(KE, 128, 4 * H)).astype(bf)
    wht = np.ascontiguousarray(Wh.T.reshape(KH, 128, 4 * H)).astype(bf)
    wzt = np.ascontiguousarray(Wz.T.reshape(KZ, 128, 2 * H)).astype(bf)
    bhi = np.ascontiguousarray((bi + bh).astype(np.float32).reshape(M4, 128).T)
    bzt = np.ascontiguousarray(bz.astype(np.float32).reshape(M2, 128).T)
    return wit, wht, wzt, bhi, bzt


def _prep_core(input_d, input_z, hidden, cell_state, t_steps):
    bf = ml_dtypes.bfloat16
    # input_d [BL, t, E] -> [E, t, BL] -> [KE, 128, t, BL]
    ind = np.ascontiguousarray(input_d.transpose(2, 1, 0)).reshape(KE, 128, t_steps, BL).astype(bf)
    inz = np.ascontiguousarray(input_z.transpose(2, 1, 0)).reshape(KZ, 128, t_steps, BL).astype(bf)
    # hidden [BL, H] -> h.T [H, BL] -> [KH, 128, BL] -> [128, KH, BL]
    h0 = np.ascontiguousarray(hidden.T.reshape(KH, 128, BL).transpose(1, 0, 2)).astype(np.float32)
    c0 = np.ascontiguousarray(cell_state.T.reshape(KH, 128, BL).transpose(1, 0, 2)).astype(np.float32)
    return ind, inz, h0, c0


def kernel(input_d, input_z, hidden, cell_state, Wi, bi, Wh, bh, Wz, bz):
    global LAST_EXEC_NS
    input_d = np.asarray(input_d, dtype=np.float32)
    input_z = np.asarray(input_z, dtype=np.float32)
    hidden = np.asarray(hidden, dtype=np.float32)
    cell_state = np.asarray(cell_state, dtype=np.float32)
    t_steps = input_d.shape[1]

    nc = _get_nc(t_steps)
    wit, wht, wzt, bhi, bzt = _prep_shared(
        np.asarray(Wi, np.float32), np.asarray(bi, np.float32),
        np.asarray(Wh, np.float32), np.asarray(bh, np.float32),
        np.asarray(Wz, np.float32), np.asarray(bz, np.float32))

    in_maps = []
    for ci in range(NCORES):
        sl = slice(ci * BL, (ci + 1) * BL)
        ind, inz, h0, c0 = _prep_core(
            input_d[sl], input_z[sl], hidden[sl], cell_state[sl], t_steps)
        in_maps.append({
            "ind": ind, "inz": inz, "wit": wit, "wht": wht, "wzt": wzt,
            "bhi": bhi, "bzt": bzt, "h0": h0, "c0": c0,
        })

    res = bass_utils.run_bass_kernel_spmd(nc, in_maps, core_ids=list(range(NCORES)))
    LAST_EXEC_NS = res.exec_time_ns

    hs = np.empty((B, t_steps, H), dtype=np.float32)
    cs = np.empty((B, t_steps, H), dtype=np.float32)
    for ci in range(NCORES):
        sl = slice(ci * BL, (ci + 1) * BL)
        # dram [128, KH, t, BL] -> [BL, t, KH, 128] -> [BL, t, H]
        hs[sl] = res.results[ci]["hs"].transpose(3, 2, 1, 0).reshape(BL, t_steps, H)
        cs[sl] = res.results[ci]["cs"].transpose(3, 2, 1, 0).reshape(BL, t_steps, H)
    return hs, cs, hs[:, -1].copy(), cs[:, -1].copy()


# revision 5
# speedup vs baseline: 1.0720x; 1.0720x over previous
"""Trainium2 Bass kernel for the LSTM-style encoder (B=256,T=256,E=512,Z=256,H=1024).

Strategy:
  - Data-parallel over batch: B=256 -> 32 rows per each of 8 cores; weights
    replicated. The sequential time loop stays local per shard.
  - Fully transposed on-chip layout: hidden/gate dim on SBUF partitions,
    batch on the free dim. Recurrent GEMM computed as gates.T = Wh @ h.T
    with Wh.T tiles (bf16) stationary, h.T (bf16) as the moving operand.
  - Input projections (Wi/Wz GEMMs) fused per time-chunk into SBUF; the
    z-path is pre-activated into u_z = sigmoid(l)*tanh(zhat) once per chunk.
  - Biases (bi+bh) folded into the gd eviction via tensor_scalar_add; bz
    folded into the z activations. Activations/elementwise in fp32.
"""

import os
import numpy as np
import ml_dtypes
from contextlib import ExitStack

import concourse.bacc as bacc
import concourse.bass as bass
import concourse.mybir as mybir
from concourse import tile
from concourse import bass_utils

BF16 = mybir.dt.bfloat16
F32 = mybir.dt.float32
AF = mybir.ActivationFunctionType

B, T, E, Z, H = 256, 256, 512, 256, 1024
NCORES = 8
BL = B // NCORES          # 32 batch rows per core
TC = 4                    # timesteps per chunk
KE, KZ, KH = E // 128, Z // 128, H // 128   # 4, 2, 8 contraction tiles
M4 = 4 * H // 128         # 32 output tiles of the 4H gate dim
M2 = 2 * H // 128         # 16 output tiles of the 2H z-gate dim

LAST_EXEC_NS = None
_CACHE = {}


def _build(t_steps=T):
    nch = t_steps // TC
    nc = bacc.Bacc("TRN2", target_bir_lowering=False, debug=False)

    ind = nc.dram_tensor("ind", [KE, 128, t_steps, BL], BF16, kind="ExternalInput").ap()
    inz = nc.dram_tensor("inz", [KZ, 128, t_steps, BL], BF16, kind="ExternalInput").ap()
    wit = nc.dram_tensor("wit", [KE, 128, 4 * H], BF16, kind="ExternalInput").ap()
    wht = nc.dram_tensor("wht", [KH, 128, 4 * H], BF16, kind="ExternalInput").ap()
    wzt = nc.dram_tensor("wzt", [KZ, 128, 2 * H], BF16, kind="ExternalInput").ap()
    bhi = nc.dram_tensor("bhi", [128, M4], F32, kind="ExternalInput").ap()
    bzt = nc.dram_tensor("bzt", [128, M2], F32, kind="ExternalInput").ap()
    h0 = nc.dram_tensor("h0", [128, KH, BL], F32, kind="ExternalInput").ap()
    c0 = nc.dram_tensor("c0", [128, KH, BL], F32, kind="ExternalInput").ap()
    hs = nc.dram_tensor("hs", [128, KH, t_steps, BL], F32, kind="ExternalOutput").ap()
    cs = nc.dram_tensor("cs", [128, KH, t_steps, BL], F32, kind="ExternalOutput").ap()

    with tile.TileContext(nc) as tc:
        with ExitStack() as ctx:
            wp = ctx.enter_context(tc.tile_pool(name="w", bufs=1))
            psum = ctx.enter_context(tc.tile_pool(name="psum", bufs=4, space="PSUM"))
            psumr = ctx.enter_context(tc.tile_pool(name="psumr", bufs=4, space="PSUM"))
            gdp = ctx.enter_context(tc.tile_pool(name="gd", bufs=2))
            uzp = ctx.enter_context(tc.tile_pool(name="uz", bufs=2))
            inp = ctx.enter_context(tc.tile_pool(name="inp", bufs=2))
            actp = ctx.enter_context(tc.tile_pool(name="act", bufs=6))
            hcp = ctx.enter_context(tc.tile_pool(name="hc", bufs=3))
            stg = ctx.enter_context(tc.tile_pool(name="stg", bufs=2))

            # ---- resident weights
            wht_sb = wp.tile([128, KH, 4 * H], BF16, tag="wht")
            for k in range(KH):
                nc.sync.dma_start(wht_sb[:, k, :], wht[k])
            wit_sb = wp.tile([128, KE, 4 * H], BF16, tag="wit")
            for k in range(KE):
                nc.sync.dma_start(wit_sb[:, k, :], wit[k])
            wzt_sb = wp.tile([128, KZ, 2 * H], BF16, tag="wzt")
            for k in range(KZ):
                nc.sync.dma_start(wzt_sb[:, k, :], wzt[k])
            bhi_sb = wp.tile([128, M4], F32, tag="bhi")
            nc.sync.dma_start(bhi_sb[:], bhi)
            bzt_sb = wp.tile([128, M2], F32, tag="bzt")
            nc.sync.dma_start(bzt_sb[:], bzt)

            # ---- carry state (h.T, c.T) in [128, KH, BL] tiling
            hT0 = hcp.tile([128, KH, BL], F32, tag="hT0")
            nc.sync.dma_start(hT0[:], h0)
            cT0 = hcp.tile([128, KH, BL], F32, tag="cT0")
            nc.sync.dma_start(cT0[:], c0)
            hbf = hcp.tile([128, KH, BL], BF16, tag="hbf")
            nc.vector.tensor_copy(hbf[:], hT0[:])
            c_prev = cT0[:]

            NF = TC * BL  # free size of one chunk (t-major, b-minor)

            for ch in range(nch):
                t0 = ch * TC
                # ---- chunk input DMA
                ind_sb = inp.tile([128, KE, TC, BL], BF16, tag="ind")
                for k in range(KE):
                    nc.sync.dma_start(ind_sb[:, k, :, :], ind[k, :, t0:t0 + TC, :])
                inz_sb = inp.tile([128, KZ, TC, BL], BF16, tag="inz")
                for k in range(KZ):
                    nc.sync.dma_start(inz_sb[:, k, :, :], inz[k, :, t0:t0 + TC, :])

                # ---- phase 1a: gd.T = Wi @ x.T + (bi+bh), per 128-row gate tile
                gd_sb = gdp.tile([128, M4, TC, BL], F32, tag="gd")
                for m in range(M4):
                    ps = psum.tile([128, TC, BL], F32, tag="ps")
                    for k in range(KE):
                        nc.tensor.matmul(
                            ps[:],
                            wit_sb[:, k, 128 * m:128 * (m + 1)],
                            ind_sb[:, k, :, :],
                            start=(k == 0), stop=(k == KE - 1),
                        )
                    nc.vector.tensor_scalar_add(gd_sb[:, m, :, :], ps[:], bhi_sb[:, m:m + 1])

                # ---- phase 1b: u_z.T = sigmoid(l.T) * tanh(zhat.T)
                uz_sb = uzp.tile([128, KH, TC, BL], F32, tag="uz")
                for j in range(KH):
                    psl = psum.tile([128, TC, BL], F32, tag="ps")
                    for k in range(KZ):
                        nc.tensor.matmul(
                            psl[:],
                            wzt_sb[:, k, 128 * j:128 * (j + 1)],
                            inz_sb[:, k, :, :],
                            start=(k == 0), stop=(k == KZ - 1),
                        )
                    sl = actp.tile([128, TC, BL], F32, tag="zact")
                    nc.scalar.activation(sl[:], psl[:], AF.Sigmoid, bias=bzt_sb[:, j:j + 1])

                    psz = psum.tile([128, TC, BL], F32, tag="ps")
                    jm = KH + j
                    for k in range(KZ):
                        nc.tensor.matmul(
                            psz[:],
                            wzt_sb[:, k, 128 * jm:128 * (jm + 1)],
                            inz_sb[:, k, :, :],
                            start=(k == 0), stop=(k == KZ - 1),
                        )
                    tz = actp.tile([128, TC, BL], F32, tag="zact")
                    nc.scalar.activation(tz[:], psz[:], AF.Tanh, bias=bzt_sb[:, jm:jm + 1])
                    nc.vector.tensor_mul(uz_sb[:, j, :, :], sl[:], tz[:])

                # ---- recurrence over the chunk
                hs_st = stg.tile([128, KH, TC, BL], F32, tag="hs_st")
                cs_st = stg.tile([128, KH, TC, BL], F32, tag="cs_st")
                for tt in range(TC):
                    # gate order [i, g, f, o]: the c-chain (needs i,g,f)
                    # finishes during the o-gate matmuls; the step tail is
                    # only add-o -> sigmoid(o) -> hbf mul.
                    gacts = {}
                    for g, fn in ((0, AF.Sigmoid), (2, AF.Tanh),
                                  (1, AF.Sigmoid), (3, AF.Sigmoid)):
                        ps = psumr.tile([128, KH, BL], F32, tag="psr")
                        for mm in range(KH):
                            m = KH * g + mm
                            for k in range(KH):
                                nc.tensor.matmul(
                                    ps[:, mm, :],
                                    wht_sb[:, k, 128 * m:128 * (m + 1)],
                                    hbf[:, k, :],
                                    start=(k == 0), stop=(k == KH - 1),
                                )
                        nc.vector.tensor_add(ps[:], ps[:], gd_sb[:, KH * g:KH * (g + 1), tt, :])
                        a = actp.tile([128, KH, BL], F32, tag="gact")
                        nc.scalar.activation(a[:], ps[:], fn)
                        gacts[g] = a

                    si, sf, tg, so = gacts[0], gacts[1], gacts[2], gacts[3]
                    tu = actp.tile([128, KH, BL], F32, tag="tmp")
                    nc.vector.tensor_mul(tu[:], si[:], tg[:])
                    u = actp.tile([128, KH, BL], F32, tag="tmp")
                    nc.vector.tensor_add(u[:], tu[:], uz_sb[:, :, tt, :])
                    t1 = actp.tile([128, KH, BL], F32, tag="tmp")
                    nc.vector.tensor_mul(t1[:], sf[:], c_prev)
                    c_ap = cs_st[:, :, tt, :]
                    nc.vector.tensor_add(c_ap, t1[:], u[:])

                    tch = actp.tile([128, KH, BL], F32, tag="tmp")
                    nc.scalar.activation(tch[:], c_ap, AF.Tanh)
                    hbf = hcp.tile([128, KH, BL], BF16, tag="hbf")
                    nc.vector.tensor_mul(hbf[:], so[:], tch[:])
                    nc.vector.tensor_mul(hs_st[:, :, tt, :], so[:], tch[:])
                    c_prev = c_ap

                nc.sync.dma_start(hs[:, :, t0:t0 + TC, :], hs_st[:])
                nc.sync.dma_start(cs[:, :, t0:t0 + TC, :], cs_st[:])

    nc.compile()
    return nc


def _get_nc(t_steps=T):
    if t_steps not in _CACHE:
        _CACHE[t_steps] = _build(t_steps)
    return _CACHE[t_steps]


def _prep_shared(Wi, bi, Wh, bh, Wz, bz):
    bf = ml_dtypes.bfloat16
    wit = np.ascontiguousarray(Wi.T.reshape(KE, 128, 4 * H)).astype(bf)
    wht = np.ascontiguousarray(Wh.T.reshape(KH, 128, 4 * H)).astype(bf)
    wzt = np.ascontiguousarray(Wz.T.reshape(KZ, 128, 2 * H)).astype(bf)
    bhi = np.ascontiguousarray((bi + bh).astype(np.float32).reshape(M4, 128).T)
    bzt = np.ascontiguousarray(bz.astype(np.float32).reshape(M2, 128).T)
    return wit, wht, wzt, bhi, bzt


def _prep_core(input_d, input_z, hidden, cell_state, t_steps):
    bf = ml_dtypes.bfloat16
    # input_d [BL, t, E] -> [E, t, BL] -> [KE, 128, t, BL]
    ind = np.ascontiguousarray(input_d.transpose(2, 1, 0)).reshape(KE, 128, t_steps, BL).astype(bf)
    inz = np.ascontiguousarray(input_z.transpose(2, 1, 0)).reshape(KZ, 128, t_steps, BL).astype(bf)
    # hidden [BL, H] -> h.T [H, BL] -> [KH, 128, BL] -> [128, KH, BL]
    h0 = np.ascontiguousarray(hidden.T.reshape(KH, 128, BL).transpose(1, 0, 2)).astype(np.float32)
    c0 = np.ascontiguousarray(cell_state.T.reshape(KH, 128, BL).transpose(1, 0, 2)).astype(np.float32)
    return ind, inz, h0, c0


def kernel(input_d, input_z, hidden, cell_state, Wi, bi, Wh, bh, Wz, bz):
    global LAST_EXEC_NS
    input_d = np.asarray(input_d, dtype=np.float32)
    input_z = np.asarray(input_z, dtype=np.float32)
    hidden = np.asarray(hidden, dtype=np.float32)
    cell_state = np.asarray(cell_state, dtype=np.float32)
    t_steps = input_d.shape[1]

    nc = _get_nc(t_steps)
    wit, wht, wzt, bhi, bzt = _prep_shared(
        np.asarray(Wi, np.float32), np.asarray(bi, np.float32),
        np.asarray(Wh, np.float32), np.asarray(bh, np.float32),
        np.asarray(Wz, np.float32), np.asarray(bz, np.float32))

    in_maps = []
    for ci in range(NCORES):
        sl = slice(ci * BL, (ci + 1) * BL)
        ind, inz, h0, c0 = _prep_core(
            input_d[sl], input_z[sl], hidden[sl], cell_state[sl], t_steps)
        in_maps.append({
            "ind": ind, "inz": inz, "wit": wit, "wht": wht, "wzt": wzt,
            "bhi": bhi, "bzt": bzt, "h0": h0, "c0": c0,
        })

    res = bass_utils.run_bass_kernel_spmd(nc, in_maps, core_ids=list(range(NCORES)))
    LAST_EXEC_NS = res.exec_time_ns

    hs = np.empty((B, t_steps, H), dtype=np.float32)
    cs = np.empty((B, t_steps, H), dtype=np.float32)
    for ci in range(NCORES):
        sl = slice(ci * BL, (ci + 1) * BL)
        # dram [128, KH, t, BL] -> [BL, t, KH, 128] -> [BL, t, H]
        hs[sl] = res.results[ci]["hs"].transpose(3, 2, 1, 0).reshape(BL, t_steps, H)
        cs[sl] = res.results[ci]["cs"].transpose(3, 2, 1, 0).reshape(BL, t_steps, H)
    return hs, cs, hs[:, -1].copy(), cs[:, -1].copy()
